# revision 30
# baseline (speedup 1.0000x reference)
"""Multi-head causal attention (B=2, T=2048, E=768, H=12, D=64) on 8 trn2 cores.

Sharding: core c handles batch b=c//4 and heads [3g, 3g+1, 3g+2] (g=c%4).
Each core computes its 3 heads' attention plus their partial contribution to the
final projection; the host sums the 4 bf16 partials per batch in fp32.

Per-core device program (bf16 operands everywhere, fp32 PSUM accumulate):
  proj:  qT/kT = Wqk^T x^T row-chunks [q0q1][k0k1][q2|k2]; k bias dropped
         (softmax-invariant). v computed directly in [s, d] layout
         (lhsT = xT chunk) into v_all[s, j, h, 0:64]; v bias is folded into a
         host-side constant output row (w_final @ b_v); ones column 64 is
         written once at setup.
  attn:  heads 0/1 interleaved so their K=64 S^T matmuls pack into PE array
         row groups (tile_position (0,0)/(64,0)) and run concurrently. S^T
         chunks pair two key-blocks side by side in one [128, 1024] 2-bank
         psum so one ACT exp covers both. Causal diag blocks are zeroed
         post-exp by a multiplicative 0/1 mask on gpsimd (sbuf-only).
         [O^T; l] accumulated via matmul(lhsT=[v_j | 1]). Normalize: 1/l via
         DVE reciprocal_approx_fast (lane-aligned on partition 64), gpsimd
         partition_broadcast, one tensor_tensor mult -> normalized O^T bf16.
  out:   out = sum_h O_h^T.T @ wf_h -> [2048, 768] bf16 partial, DMA out.

`repeat` unrolls the whole body N times in one NEFF; used by test.py to
measure per-body HW time as slope(t vs N), cancelling dispatch overhead.
"""
import numpy as np

EMBED_DIM = 768
B = 2
T = 2048
N_CORES = 8
NT = T // 128           # 16 query/key tiles
SCALE = 1.0 / np.sqrt(64.0)
NEG = -1.0e9

_state = {}


def _build(repeat=1):
    import concourse.tile as tile
    from concourse import bacc, mybir

    F32 = mybir.dt.float32
    F32R = mybir.dt.float32r
    BF16 = mybir.dt.bfloat16

    nc = bacc.Bacc("TRN2", target_bir_lowering=False, debug=False)

    xT_d = nc.dram_tensor("xT", [EMBED_DIM, T], BF16, kind="ExternalInput").ap()
    # columns ordered [q0 q1 | k0 k1 | q2 | k2]
    wqk_d = nc.dram_tensor("wqk", [EMBED_DIM, 384], BF16, kind="ExternalInput").ap()
    wv_d = nc.dram_tensor("wv", [EMBED_DIM, 192], BF16, kind="ExternalInput").ap()
    bqA_d = nc.dram_tensor("bqA", [128, 1], F32, kind="ExternalInput").ap()
    bqB_d = nc.dram_tensor("bqB", [128, 1], F32, kind="ExternalInput").ap()
    wf_d = nc.dram_tensor("wf", [192, EMBED_DIM], BF16, kind="ExternalInput").ap()
    # 0/1 lower-tri mask, duplicated twice along dim 1 (multiplies P post-exp)
    mask_d = nc.dram_tensor("mask2", [128, 256], BF16, kind="ExternalInput").ap()
    out_d = nc.dram_tensor("out_p", [T, EMBED_DIM], BF16, kind="ExternalOutput").ap()
    import os
    dbg = os.environ.get("KDEBUG")
    dbg_v_d = dbg_o_d = None
    if dbg:
        dbg_v_d = nc.dram_tensor("dbg_v", [128, NT * 3 * 65], BF16,
                                 kind="ExternalOutput").ap()
        dbg_o_d = nc.dram_tensor("dbg_o", [128, T], BF16,
                                 kind="ExternalOutput").ap()
        dbg_r_d = nc.dram_tensor("dbg_r", [1, 512], F32,
                                 kind="ExternalOutput").ap()
        dbg_b_d = nc.dram_tensor("dbg_b", [64, 512], F32,
                                 kind="ExternalOutput").ap()
        dbg_l_d = nc.dram_tensor("dbg_l", [128, 512], F32,
                                 kind="ExternalOutput").ap()

    with tile.TileContext(nc) as tc:
        with tc.tile_pool(name="const", bufs=1) as const, \
             tc.tile_pool(name="persist", bufs=1) as persist:
            # ---- constants ----
            wqk_sb = const.tile([128, 6, 384], BF16)
            wv_sb = const.tile([128, 6, 192], BF16)
            nc.sync.dma_start(out=wqk_sb[:],
                              in_=wqk_d.rearrange("(k p) c -> p k c", p=128))
            nc.sync.dma_start(out=wv_sb[:],
                              in_=wv_d.rearrange("(k p) c -> p k c", p=128))
            bqA_sb = const.tile([128, 1], F32)
            bqB_sb = const.tile([128, 1], F32)
            mask_sb = const.tile([128, 2, 128], BF16)
            nc.sync.dma_start(out=bqA_sb[:], in_=bqA_d)
            nc.sync.dma_start(out=bqB_sb[:], in_=bqB_d)
            nc.sync.dma_start(out=mask_sb[:],
                              in_=mask_d.rearrange("p (b c) -> p b c", b=2))
            wf01_sb = const.tile([128, EMBED_DIM], BF16)
            wf2_sb = const.tile([64, EMBED_DIM], BF16)
            nc.sync.dma_start(out=wf01_sb[:], in_=wf_d[0:128, :])
            nc.sync.dma_start(out=wf2_sb[:], in_=wf_d[128:192, :])


            # ---- persistent activations (x2: ping-pong across bodies) ----
            pers = []
            for i in range(min(repeat, 2)):
                p = {
                    "qA": persist.tile([128, T], BF16, name=f"qA{i}"),
                    "kA": persist.tile([128, T], BF16, name=f"kA{i}"),
                    # q2 dup'd on both partition halves (S row-group packing)
                    "qB": persist.tile([128, T], BF16, name=f"qB{i}"),
                    "kB": persist.tile([128, T], BF16, name=f"kB{i}"),
                    "v_all": persist.tile([128, NT, 3, 65], BF16,
                                          name=f"v_all{i}"),
                    "ot01": persist.tile([128, T], BF16, name=f"ot01{i}"),
                    "ot2": persist.tile([64, T], BF16, name=f"ot2{i}"),
                }
                nc.vector.memset(p["v_all"][:, :, :, 64:65], 1.0)
                pers.append(p)

            for rep in range(repeat):
                _emit_body(nc, tc, rep, pers[rep % len(pers)], locals())
            if dbg:
                nc.sync.dma_start(
                    out=dbg_v_d,
                    in_=pers[0]["v_all"][:].rearrange("p a b c -> p (a b c)"))
                nc.sync.dma_start(out=dbg_o_d, in_=pers[0]["ot01"][:])

    nc.compile()
    return nc


def _chunk_items(q):
    """Key-block items of query-quarter q grouped into <=1024-col chunks."""
    base = 512 * q
    items = []
    for j in range(4 * q + 4):
        s0 = max(base, 128 * j)
        items.append((j, s0, base + 512 - s0))
    chunks, cur, tot = [], [], 0
    for it in items:
        if tot + it[2] > 512:
            chunks.append(cur)
            cur, tot = [], 0
        cur.append(it)
        tot += it[2]
    chunks.append(cur)
    return chunks


def _emit_body(nc, tc, rep, pers, env):
    """Emit one forward pass. Projections/v/phase-3 tiles are drip-fed into
    the attention chunk stream via `filler` so the PE never starves while
    ACT works through the exps.

    PSUM (8 banks): ps0 ps1 (proj groups + v tiles + phase-3, 1 bank each),
    st0 st1 ([128,1024] S^T chunks, 2 banks each), otl0 otl1 ([O^T; l] accum +
    in-bank recip broadcast, 1 bank each)."""
    from collections import deque

    from concourse import mybir

    F32 = mybir.dt.float32
    F32R = mybir.dt.float32r
    BF16 = mybir.dt.bfloat16
    Exp = mybir.ActivationFunctionType.Exp
    ADD = mybir.AluOpType.add
    MULT = mybir.AluOpType.mult

    xT_d, out_d = env["xT_d"], env["out_d"]
    wqk_sb, wv_sb = env["wqk_sb"], env["wv_sb"]
    bqA_sb, bqB_sb = env["bqA_sb"], env["bqB_sb"]
    wf01_sb, wf2_sb = env["wf01_sb"], env["wf2_sb"]
    mask_sb = env["mask_sb"]
    qA, kA, qB, kB = pers["qA"], pers["kA"], pers["qB"], pers["kB"]
    v_all = pers["v_all"]
    ot01, ot2 = pers["ot01"], pers["ot2"]

    with tc.tile_pool(name=f"sb{rep}", bufs=1) as sbp, \
         tc.tile_pool(name=f"ps{rep}", bufs=1, space="PSUM") as psp:
        # ---- input DMA: xT as 12 [128, 1024] chunks, first half first ----
        xk = sbp.tile([128, 6, 2, 1024], BF16, name=f"xk{rep}", tag="xk")
        for hf in range(2):
            for k in range(6):
                nc.sync.dma_start(
                    out=xk[:, k, hf, :],
                    in_=xT_d[128 * k:128 * (k + 1), 1024 * hf:1024 * (hf + 1)])

        psidx = [0]

        def qk_group(m, n):
            # m: 0=[q0q1]->qA(+bias), 1=[k0k1]->kA, 2=[q2|k2]->qB(+bias)/kB
            c0 = 128 * m
            ps = psp.tile([128, 512], F32, name=f"pg{rep}_{m}_{n}",
                          tag=f"ps{psidx[0] % 2}")
            psidx[0] += 1
            hf, off = divmod(512 * n, 1024)
            for k in range(6):
                nc.tensor.matmul(ps[:], lhsT=wqk_sb[:, k, c0:c0 + 128],
                                 rhs=xk[:, k, hf, off:off + 512],
                                 start=(k == 0), stop=(k == 5))
            nsl = slice(512 * n, 512 * (n + 1))
            if m == 0:
                nc.vector.tensor_scalar_add(out=qA[:, nsl], in0=ps[:],
                                            scalar1=bqA_sb[:])
            elif m == 1:
                nc.scalar.copy(out=kA[:, nsl], in_=ps[:])
            else:
                nc.vector.tensor_scalar_add(out=qB[0:64, nsl], in0=ps[0:64, :],
                                            scalar1=bqB_sb[0:64, :])
                nc.scalar.copy(out=kB[0:64, nsl], in_=ps[64:128, :])

        def v_group(i):
            vp = psp.tile([128, 192], F32, name=f"vp{rep}_{i}",
                          tag=f"ps{psidx[0] % 2}")
            psidx[0] += 1
            hf, off = divmod(128 * i, 1024)
            for k in range(6):
                nc.tensor.matmul(vp[:], lhsT=xk[:, k, hf, off:off + 128],
                                 rhs=wv_sb[:, k, :], start=(k == 0), stop=(k == 5))
            # v bias is folded into the host-side output correction; pure copy
            if i % 2 == 0:
                nc.vector.tensor_copy(out=v_all[:, i, :, 0:64],
                                      in_=vp[:].rearrange("p (g d) -> p g d", g=3))
            else:
                nc.scalar.copy(out=v_all[:, i, :, 0:64],
                               in_=vp[:].rearrange("p (g d) -> p g d", g=3))

        def attn(q, heads, filler):
            """One query-quarter for `heads` ((0,1) packed pair, or (2,)).

            Chunk loop is software-pipelined: O-matmuls for chunk c are
            emitted after S/exp of chunk c+1, so the in-order PE queue can
            stream S(c+1) while ACT works through exp(c)."""
            base = 512 * q
            chunks = _chunk_items(q)
            otls = {}
            for h in heads:
                otls[h] = psp.tile([128, 512], F32, name=f"otl{rep}_{h}{q}",
                                   tag=f"otl{(h if len(heads) == 2 else q) % 2}")

            def emit_o(ci, ch):
                for h in heads:
                    col = 0
                    for (j, s0, ln) in ch:
                        nc.tensor.matmul(
                            otls[h][0:65, s0 - base:s0 - base + ln],
                            lhsT=v_all[:, j, h, :],
                            rhs=pts[ci][h][:, col:col + ln],
                            start=(ci == 0 and j == ch[0][0]),
                            stop=(ci == len(chunks) - 1 and j == ch[-1][0]),
                            skip_group_check=True)
                        col += ln

            pts = {}
            for ci, ch in enumerate(chunks):
                tot = sum(ln for (_, _, ln) in ch)
                sts = {}
                pts[ci] = {}
                for h in heads:
                    sts[h] = psp.tile([128, 512], F32,
                                      name=f"st{rep}_{h}{q}{ci}",
                                      tag=f"st{h % 2}{ci % 2}")
                    pts[ci][h] = sbp.tile([128, 512], BF16,
                                          name=f"pt{rep}_{h}{q}{ci}",
                                          tag=f"pt{h}{ci % 3}")
                col = 0
                for (j, s0, ln) in ch:
                    for h in heads:
                        o = 64 * ((h if h < 2 else 0) % 2)
                        kT = kA if h < 2 else kB
                        qT = qA if h < 2 else qB
                        nc.tensor.matmul(
                            sts[h][:, col:col + ln],
                            lhsT=kT[o:o + 64, 128 * j:128 * (j + 1)],
                            rhs=qT[o:o + 64, s0:s0 + ln],
                            start=True, stop=True, tile_position=(o, 0))
                    col += ln
                for h in heads:
                    nc.scalar.activation(out=pts[ci][h][:, 0:tot],
                                         in_=sts[h][:, 0:tot],
                                         func=Exp, scale=float(SCALE))
                # zero masked (upper-tri) entries of diag blocks post-exp
                diag_cols = []
                col = 0
                for (j, s0, ln) in ch:
                    if s0 == 128 * j:
                        diag_cols.append(col)
                    col += ln
                for h in heads:
                    if len(diag_cols) == 2:
                        stride = diag_cols[1]
                        pm = pts[ci][h][:, 0:2 * stride].rearrange(
                            "p (b c) -> p b c", c=stride)[:, 0:2, 0:128]
                        nc.gpsimd.tensor_tensor(out=pm, in0=pm,
                                                in1=mask_sb[:], op=MULT)
                    elif len(diag_cols) == 1:
                        pm = pts[ci][h][:, 0:128]
                        nc.gpsimd.tensor_tensor(out=pm, in0=pm,
                                                in1=mask_sb[:, 0, :], op=MULT)
                if ci > 0:
                    emit_o(ci - 1, chunks[ci - 1])
                filler()
            emit_o(len(chunks) - 1, chunks[-1])
            for h in heads:
                otl = otls[h]
                lsb = sbp.tile([1, 512], F32, name=f"ls{rep}_{h}{q}",
                               tag=f"ls{h}")
                nc.vector.tensor_copy(out=lsb[:], in_=otl[64:65, 0:512])
                rsb = sbp.tile([1, 512], F32, name=f"rs{rep}_{h}{q}",
                               tag=f"rs{h}")
                nc.vector.reciprocal_approx_fast(out=rsb[:], in_=lsb[:])
                bcs = sbp.tile([64, 512], F32, name=f"bc{rep}_{h}{q}",
                               tag=f"bc{h}")
                nc.gpsimd.partition_broadcast(bcs[:], rsb[:])
                if env.get("dbg") and rep == 0 and h == 0 and q == 3:
                    lsb = sbp.tile([128, 512], F32, name="dbg_lsb", tag="dbgl")
                    nc.vector.tensor_copy(out=lsb[:], in_=otl[:])
                    nc.sync.dma_start(out=env["dbg_r_d"], in_=rsb[:])
                    nc.sync.dma_start(out=env["dbg_b_d"], in_=bcs[:])
                    nc.sync.dma_start(out=env["dbg_l_d"], in_=lsb[:])
                if h == 2:
                    dst = ot2[0:64, base:base + 512]
                else:
                    dst = ot01[64 * h:64 * h + 64, base:base + 512]
                nc.vector.tensor_tensor(out=dst, in0=otl[0:64, :],
                                        in1=bcs[:], op=MULT)

        def phase3(i):
            fpa = psp.tile([128, 512], F32, name=f"fpa{rep}_{i}",
                           tag=f"ps{psidx[0] % 2}")
            psidx[0] += 1
            fpb = psp.tile([128, 256], F32, name=f"fpb{rep}_{i}",
                           tag=f"ps{psidx[0] % 2}")
            psidx[0] += 1
            for (fp, n0, n1) in ((fpa, 0, 512), (fpb, 512, 768)):
                nc.tensor.matmul(fp[:, 0:n1 - n0],
                                 lhsT=ot01[:, 128 * i:128 * (i + 1)],
                                 rhs=wf01_sb[:, n0:n1], start=True, stop=False)
                nc.tensor.matmul(fp[:, 0:n1 - n0],
                                 lhsT=ot2[0:64, 128 * i:128 * (i + 1)],
                                 rhs=wf2_sb[:, n0:n1], start=False, stop=True)
            ob = sbp.tile([128, EMBED_DIM], BF16, name=f"ob{rep}_{i}",
                          tag=f"ob{i % 3}")
            nc.scalar.copy(out=ob[:, 0:512], in_=fpa[:])
            nc.vector.tensor_copy(out=ob[:, 512:768], in_=fpb[:])
            nc.sync.dma_start(out=out_d[128 * i:128 * (i + 1), :], in_=ob[:])

        # ---- emission schedule ----
        import os
        kphases = os.environ.get("KPHASES", "123")

        qk_group(0, 0)
        qk_group(1, 0)
        for i in range(4):
            v_group(i)
        qk_group(0, 1)
        qk_group(1, 1)
        for i in range(4, 8):
            v_group(i)

        pending = deque()
        pending.extend([lambda: qk_group(0, 2), lambda: qk_group(1, 2)])
        pending.extend([(lambda i=i: v_group(i)) for i in range(8, 12)])
        pending.extend([lambda: qk_group(0, 3), lambda: qk_group(1, 3)])
        pending.extend([(lambda i=i: v_group(i)) for i in range(12, 16)])
        pending.extend([(lambda n=n: qk_group(2, n)) for n in range(4)])

        def filler():
            if pending:
                pending.popleft()()

        if "2" in kphases:
            for q in range(4):
                attn(q, (0, 1), filler)
        while pending:
            pending.popleft()()

        if "2" in kphases:
            p3 = deque()
            for q in range(4):
                attn(q, (2,), lambda: (p3.popleft()() if p3 else None))
                if "3" in kphases:
                    p3.extend([(lambda i=i: phase3(i))
                               for i in range(4 * q, 4 * q + 4)])
            while p3:
                p3.popleft()()
        elif "3" in kphases:
            for i in range(NT):
                phase3(i)


def _prep_inputs(x, w_qkv, b_qkv, w_final):
    """Build the 8 per-core input maps from the full inputs."""
    import ml_dtypes
    bf16 = ml_dtypes.bfloat16

    x = np.asarray(x, dtype=np.float32)
    w_qkv = np.asarray(w_qkv, dtype=np.float32)
    b_qkv = np.asarray(b_qkv, dtype=np.float32)
    w_final = np.asarray(w_final, dtype=np.float32)
    E = EMBED_DIM

    tri = (np.arange(128)[:, None] <= np.arange(128)[None, :]).astype(bf16)
    mask2 = np.ascontiguousarray(np.concatenate([tri, tri], axis=1))  # [128,256]
    in_maps = []
    for c in range(N_CORES):
        b = c // 4
        g = c % 4
        heads = [3 * g, 3 * g + 1, 3 * g + 2]
        hr = [np.arange(64 * h, 64 * h + 64) for h in heads]
        # [q0 q1 | k0 k1 | q2 | k2]
        rows_qk = np.concatenate([hr[0], hr[1], E + hr[0], E + hr[1],
                                  hr[2], E + hr[2]])
        rows_v = np.concatenate(hr) + 2 * E
        xT = np.ascontiguousarray(x[b].T).astype(bf16)               # [768, 2048]
        wqk = np.ascontiguousarray(w_qkv[rows_qk].T).astype(bf16)    # [768, 384]
        wv = np.ascontiguousarray(w_qkv[rows_v].T).astype(bf16)      # [768, 192]
        bqA = np.ascontiguousarray(
            b_qkv[np.concatenate([hr[0], hr[1]])][:, None])          # [128, 1]
        bqB = np.ascontiguousarray(
            np.concatenate([b_qkv[hr[2]], np.zeros(64, np.float32)])[:, None])
        wf = np.ascontiguousarray(
            w_final[:, np.concatenate(hr)].T).astype(bf16)           # [192, 768]
        in_maps.append({"xT": xT, "wqk": wqk, "wv": wv, "bqA": bqA,
                        "bqB": bqB, "wf": wf, "mask2": mask2})
    return in_maps


def kernel(x, w_qkv, b_qkv, w_final, _trace=False):
    from concourse.bass_utils import run_bass_kernel_spmd

    if "nc" not in _state:
        _state["nc"] = _build()
    nc = _state["nc"]

    in_maps = _prep_inputs(x, w_qkv, b_qkv, w_final)
    res = run_bass_kernel_spmd(nc, in_maps, list(range(N_CORES)), trace=_trace)
    _state["last_result"] = res

    # v-bias contribution to the output is a constant row: w_final @ b_v
    bias_row = (np.asarray(w_final, np.float32)
                @ np.asarray(b_qkv, np.float32)[2 * EMBED_DIM:])
    out = np.empty((B, T, EMBED_DIM), dtype=np.float32)
    for b in range(B):
        acc = np.tile(bias_row[None, :], (T, 1))
        for g in range(4):
            acc += res.results[4 * b + g]["out_p"].astype(np.float32)
        out[b] = acc
    return out
